# revision 1
# baseline (speedup 1.0000x reference)
"""v11 = v8 + fast-start front phase: x loads/transposes first, w_qkv split
into per-section tiles (wq/wk/wv) so qkT(0)/qkT(8) only depend on their own
weight DMAs, V production interleaved into pair 0's jt loop, w_proj loaded
after attention starts. exp stream should begin ~30-40us earlier.
"""
import sys

sys.path.insert(0, "/opt/trn_rl_repo")

import numpy as np

N = 1024
D = 1024
H = 16
HD = 64
SCALE = HD ** -0.5
P = 128
NT = N // P
DTn = D // P
HC = 512
VW = 65

_CACHE: dict = {}


def _build_nc(repeat=1):
    import concourse.bass as bass
    import concourse.tile as tile
    from concourse import bacc, mybir
    from concourse.masks import make_identity
    from contextlib import ExitStack

    fp32 = mybir.dt.float32
    bf16 = mybir.dt.bfloat16
    EXP = mybir.ActivationFunctionType.Exp

    nc = bacc.Bacc("TRN2", debug=False, num_devices=8)
    x_d = nc.dram_tensor("x", [N, D], fp32, kind="ExternalInput").ap()
    wqkv_d = nc.dram_tensor("w_qkv", [D, 3 * D], fp32, kind="ExternalInput").ap()
    wproj_d = nc.dram_tensor("w_proj", [D, D], fp32, kind="ExternalInput").ap()
    b_d = nc.dram_tensor("b_proj", [1, D], fp32, kind="ExternalInput").ap()
    out_d = nc.dram_tensor("out", [N, D], fp32, kind="ExternalOutput").ap()

    with tile.TileContext(nc) as tc, ExitStack() as ctx:
        stg = ctx.enter_context(tc.tile_pool(name="stg", bufs=3))
        xbfp = ctx.enter_context(tc.tile_pool(name="xbf", bufs=2))
        xTp = ctx.enter_context(tc.tile_pool(name="xT", bufs=NT))
        wqp = ctx.enter_context(tc.tile_pool(name="wq", bufs=DTn))
        wkp = ctx.enter_context(tc.tile_pool(name="wk", bufs=DTn))
        wvp = ctx.enter_context(tc.tile_pool(name="wv", bufs=DTn))
        wprojp = ctx.enter_context(tc.tile_pool(name="wproj", bufs=DTn))
        qkTp = ctx.enter_context(tc.tile_pool(name="qkT", bufs=2 * NT))
        vp = ctx.enter_context(tc.tile_pool(name="vsb", bufs=NT))
        eTp = ctx.enter_context(tc.tile_pool(name="eT", bufs=5))
        attnp = ctx.enter_context(tc.tile_pool(name="attnT", bufs=2 * NT))
        avsbp = ctx.enter_context(tc.tile_pool(name="avsb", bufs=3))
        recp = ctx.enter_context(tc.tile_pool(name="rec", bufs=3))
        rbp = ctx.enter_context(tc.tile_pool(name="rb", bufs=3))
        constp = ctx.enter_context(tc.tile_pool(name="const", bufs=1))
        ysbp = ctx.enter_context(tc.tile_pool(name="ysb", bufs=2))
        psS = ctx.enter_context(tc.tile_pool(name="psS", bufs=3, space="PSUM"))
        psAcc = ctx.enter_context(tc.tile_pool(name="psAcc", bufs=2, space="PSUM"))

        for _rep in range(repeat):
            ident = constp.tile([P, P], bf16, name="ident", tag="ident")
            make_identity(nc, ident[:])
            ones_row = constp.tile([1, P], bf16, name="ones_row", tag="ones_row")
            nc.gpsimd.memset(ones_row[:], 1.0)
            b_stage = constp.tile([1, D], fp32, name="b_stage", tag="b_stage")
            nc.sync.dma_start(b_stage[:], b_d[:])
            b_bf = constp.tile([1, D], bf16, name="b_bf", tag="b_bf")
            nc.vector.tensor_copy(b_bf[:], b_stage[:])

            # --- x first: load, convert (ACT), PE-transpose ---
            xT = [xTp.tile([P, N], bf16, name=f"xT{t}", tag="xT")
                  for t in range(DTn)]
            for it in range(NT):
                s = stg.tile([P, 1024], fp32, name="stg_t", tag="stg")
                nc.sync.dma_start(s[:], x_d[it * P:(it + 1) * P, :])
                xb = xbfp.tile([P, D], bf16, name="xb_t", tag="xb")
                nc.scalar.copy(xb[:], s[:])
                for dt in range(DTn):
                    pt = psS.tile([P, P], bf16, name="pst_t", tag="pss")
                    nc.tensor.transpose(pt[:], xb[:, dt * P:(dt + 1) * P],
                                        ident[:])
                    nc.vector.tensor_copy(xT[dt][:, it * P:(it + 1) * P], pt[:])

            # --- weights as separate Q/K/V section tiles ---
            def load_w(dst_pool, src_col, name):
                tiles = []
                for t in range(DTn):
                    s = stg.tile([P, 1024], fp32, name="stg_t", tag="stg")
                    nc.sync.dma_start(
                        s[:], wqkv_d[t * P:(t + 1) * P,
                                     src_col:src_col + 1024])
                    w = dst_pool.tile([P, 1024], bf16, name=f"{name}{t}",
                                      tag=name)
                    nc.vector.tensor_copy(w[:], s[:])
                    tiles.append(w)
                return tiles

            wq = load_w(wqp, 0, "wq")
            wk = load_w(wkp, 1024, "wk")

            qkT = [None] * (2 * NT)

            def emit_qkT(ft):
                wt = wq if ft < NT else wk
                col = (ft % NT) * P
                ps = psS.tile([P, N], fp32, name="pss_t", tag="pss")
                for dt in range(DTn):
                    for ic in range(2):
                        nc.tensor.matmul(
                            ps[:, ic * HC:(ic + 1) * HC],
                            lhsT=wt[dt][:, col:col + P],
                            rhs=xT[dt][:, ic * HC:(ic + 1) * HC],
                            start=(dt == 0), stop=(dt == DTn - 1))
                q = qkTp.tile([P, N], bf16, name=f"qkT{ft}", tag="qkT")
                nc.vector.tensor_copy(q[:], ps[:])
                qkT[ft] = q

            emit_qkT(0)
            emit_qkT(NT)

            wv = load_w(wvp, 2048, "wv")
            vsb = [None] * NT

            def emit_v(jt):
                ps = psS.tile([P, N], fp32, name="pss_t", tag="pss")
                for dt in range(DTn):
                    for dc in range(2):
                        nc.tensor.matmul(
                            ps[:, dc * HC:(dc + 1) * HC],
                            lhsT=xT[dt][:, jt * P:(jt + 1) * P],
                            rhs=wv[dt][:, dc * HC:(dc + 1) * HC],
                            start=(dt == 0), stop=(dt == DTn - 1))
                v = vp.tile([P, H * VW], bf16, name=f"v{jt}", tag="v")
                nc.gpsimd.memset(v[:], 1.0)
                vv = v[:].rearrange("p (h c) -> p h c", c=VW)
                pv = ps[:].rearrange("p (h c) -> p h c", c=HD)
                nc.vector.tensor_copy(vv[:, :, 0:HD], pv)
                vsb[jt] = v

            emit_v(0)
            emit_v(1)

            attnT = [[attnp.tile([P, HC], bf16, name=f"attnT{c}_{q}",
                                 tag="attnT") for q in range(NT)]
                     for c in range(2)]

            def attend_pair(hp, icb, jt_hook=None):
                ha, hb = 2 * hp, 2 * hp + 1
                qa, ka = qkT[hp], qkT[NT + hp]
                i0 = icb * HC
                av = {h: psAcc.tile([VW, HC], fp32, name=f"av{h}_{icb}",
                                    tag="av") for h in (ha, hb)}
                for jt in range(NT):
                    ps = psS.tile([P, N], fp32, name="pss_t", tag="pss")
                    nc.tensor.matmul(
                        ps[:, 0:HC],
                        lhsT=ka[0:HD, jt * P:(jt + 1) * P],
                        rhs=qa[0:HD, i0:i0 + HC],
                        start=True, stop=True)
                    nc.tensor.matmul(
                        ps[:, HC:N],
                        lhsT=ka[HD:P, jt * P:(jt + 1) * P],
                        rhs=qa[HD:P, i0:i0 + HC],
                        start=True, stop=True)
                    e = eTp.tile([P, N], bf16, name=f"e{hp}_{jt}", tag="e")
                    nc.scalar.activation(e[:], ps[:], EXP, scale=SCALE)
                    if jt_hook is not None:
                        jt_hook(jt)
                    for h, ec in ((ha, 0), (hb, HC)):
                        nc.tensor.matmul(
                            av[h][:],
                            lhsT=vsb[jt][:, h * VW:(h + 1) * VW],
                            rhs=e[:, ec:ec + HC],
                            start=(jt == 0), stop=(jt == NT - 1))
                for h, off in ((ha, 0), (hb, HD)):
                    avs = avsbp.tile([VW, HC], bf16, name=f"avs{h}", tag="avs")
                    nc.vector.tensor_copy(avs[:], av[h][:])
                    recf = recp.tile([1, HC], fp32, name=f"recf{h}", tag="recf")
                    nc.vector.reciprocal(recf[:], avs[HD:VW, :])
                    rb = rbp.tile([HD, HC], fp32, name=f"rb{h}", tag="rb")
                    nc.gpsimd.partition_broadcast(rb[:], recf[:])
                    nc.vector.tensor_mul(attnT[icb][hp][off:off + HD, :],
                                         avs[0:HD, :], rb[:])

            def proj_tile(it):
                icb = it // 4
                ps = psS.tile([P, N], fp32, name="pss_t", tag="pss")
                for fc in range(2):
                    for dt in range(DTn):
                        nc.tensor.matmul(
                            ps[:, fc * HC:(fc + 1) * HC],
                            lhsT=attnT[icb][dt][:, (it % 4) * P:
                                                (it % 4 + 1) * P],
                            rhs=wproj[dt][:, fc * HC:(fc + 1) * HC],
                            start=(dt == 0), stop=False)
                    nc.tensor.matmul(
                        ps[:, fc * HC:(fc + 1) * HC],
                        lhsT=ones_row[0:1, :],
                        rhs=b_bf[0:1, fc * HC:(fc + 1) * HC],
                        start=False, stop=True)
                y = ysbp.tile([P, N], fp32, name="y_t", tag="y")
                nc.vector.tensor_copy(y[:], ps[:])
                nc.sync.dma_start(out_d[it * P:(it + 1) * P, :], y[:])

            # pair 0 chunk 0: V(2..7) emitted inside the jt loop, one ahead
            def v_hook(jt):
                if jt + 2 < NT and vsb[jt + 2] is None:
                    emit_v(jt + 2)

            attend_pair(0, 0, jt_hook=v_hook)

            # w_proj loads after attention is underway
            wproj = []
            for t in range(DTn):
                s = stg.tile([P, 1024], fp32, name="stg_t", tag="stg")
                nc.sync.dma_start(s[:], wproj_d[t * P:(t + 1) * P, :])
                w = wprojp.tile([P, D], bf16, name=f"wproj{t}", tag="wproj")
                nc.vector.tensor_copy(w[:], s[:])
                wproj.append(w)

            for hp in range(1, NT):
                if qkT[hp] is None:
                    emit_qkT(hp)
                    emit_qkT(NT + hp)
                if hp + 1 < NT and qkT[hp + 1] is None:
                    emit_qkT(hp + 1)
                    emit_qkT(NT + hp + 1)
                attend_pair(hp, 0)
            for hp in range(NT):
                attend_pair(hp, 1)
                if hp >= 4:
                    proj_tile(hp - 4)
            for it in range(4, NT):
                proj_tile(it)

    nc.compile()
    return nc


def get_nc():
    if "nc" not in _CACHE:
        _CACHE["nc"] = _build_nc()
    return _CACHE["nc"]


def kernel(x, w_qkv, w_proj, b_proj):
    from concourse import bass_utils

    nc = get_nc()
    x = np.ascontiguousarray(x, dtype=np.float32)
    w_qkv = np.ascontiguousarray(w_qkv, dtype=np.float32)
    w_proj = np.ascontiguousarray(w_proj, dtype=np.float32)
    b2 = np.ascontiguousarray(b_proj, dtype=np.float32).reshape(1, D)
    in_maps = [
        {"x": x[i], "w_qkv": w_qkv, "w_proj": w_proj, "b_proj": b2}
        for i in range(8)
    ]
    res = bass_utils.run_bass_kernel_spmd(nc, in_maps, core_ids=list(range(8)))
    return np.stack([res.results[i]["out"] for i in range(8)], axis=0)



# revision 7
# speedup vs baseline: 1.1021x; 1.1021x over previous
"""v12: host-prepped bf16 inputs (xT pre-transposed, w split/converted) kill
all on-device conversions/transposes; single software-pipelined attention
stream (lag-1 scores->exp->AV) with qkv/v/proj emission chunks pumped as PE
filler so the exp latency is never exposed; bias add moved to DVE; PE warmup
matmuls ramp the clock before the first real work.
"""
import sys

sys.path.insert(0, "/opt/trn_rl_repo")

import numpy as np

N = 1024
D = 1024
H = 16
HD = 64
SCALE = HD ** -0.5
P = 128
NT = N // P          # 8 token tiles
DTn = D // P         # 8 dim tiles
HC = 512             # half-row chunk (one psum bank of fp32)
VW = 65              # v width per head: 64 dims + ones column (denominator)

_CACHE: dict = {}


def _build_nc(repeat=1):
    import concourse.bass as bass  # noqa: F401
    import concourse.tile as tile
    from concourse import bacc, mybir
    from contextlib import ExitStack

    fp32 = mybir.dt.float32
    bf16 = mybir.dt.bfloat16
    EXP = mybir.ActivationFunctionType.Exp

    nc = bacc.Bacc("TRN2", debug=False, num_devices=8)
    xT_d = nc.dram_tensor("xT", [D, N], bf16, kind="ExternalInput").ap()
    wq_d = nc.dram_tensor("wq", [D, D], bf16, kind="ExternalInput").ap()
    wk_d = nc.dram_tensor("wk", [D, D], bf16, kind="ExternalInput").ap()
    wv_d = nc.dram_tensor("wv", [D, D], bf16, kind="ExternalInput").ap()
    wp_d = nc.dram_tensor("wp", [D, D], bf16, kind="ExternalInput").ap()
    b_d = nc.dram_tensor("b_proj", [1, D], bf16, kind="ExternalInput").ap()
    out_d = nc.dram_tensor("out", [N, D], bf16, kind="ExternalOutput").ap()

    with tile.TileContext(nc) as tc, ExitStack() as ctx:
        constp = ctx.enter_context(tc.tile_pool(name="const", bufs=1))
        xTp = ctx.enter_context(tc.tile_pool(name="xT", bufs=DTn))
        wqp = ctx.enter_context(tc.tile_pool(name="wq", bufs=DTn))
        wkp = ctx.enter_context(tc.tile_pool(name="wk", bufs=DTn))
        wvp = ctx.enter_context(tc.tile_pool(name="wv", bufs=DTn))
        wpp = ctx.enter_context(tc.tile_pool(name="wp", bufs=DTn))
        qkTp = ctx.enter_context(tc.tile_pool(name="qkT", bufs=2 * NT))
        vp = ctx.enter_context(tc.tile_pool(name="vsb", bufs=NT))
        eTp = ctx.enter_context(tc.tile_pool(name="eT", bufs=4))
        attnp = ctx.enter_context(tc.tile_pool(name="attnT", bufs=2 * NT))
        avsp = ctx.enter_context(tc.tile_pool(name="avs", bufs=3))
        recp = ctx.enter_context(tc.tile_pool(name="rec", bufs=3))
        rbp = ctx.enter_context(tc.tile_pool(name="rb", bufs=3))
        ysbp = ctx.enter_context(tc.tile_pool(name="ysb", bufs=2))
        psSp = ctx.enter_context(tc.tile_pool(name="psS", bufs=2, space="PSUM"))
        psAp = ctx.enter_context(tc.tile_pool(name="psA", bufs=2, space="PSUM"))
        psEp = ctx.enter_context(tc.tile_pool(name="psE", bufs=2, space="PSUM"))

        for _rep in range(repeat):
            # ---- constants / warmup ----
            warm = constp.tile([P, HC], bf16, name="warm", tag="warm")
            nc.gpsimd.memset(warm[:], 0.0)
            for _w in range(3):
                pw = psEp.tile([P, HC], fp32, name="pw", tag="em")
                nc.tensor.matmul(pw[:], lhsT=warm[:, 0:P], rhs=warm[:],
                                 start=True, stop=True)

            b_sb = constp.tile([1, D], bf16, name="b_sb", tag="b_sb")
            nc.sync.dma_start(b_sb[:], b_d[:])
            bias_bc = constp.tile([P, D], bf16, name="bias_bc", tag="bias_bc")
            nc.gpsimd.partition_broadcast(bias_bc[:], b_sb[:])

            # ---- input DMAs (order = arrival priority) ----
            xT = [xTp.tile([P, N], bf16, name=f"xT{t}", tag="xT")
                  for t in range(DTn)]
            wqs = [wqp.tile([P, D], bf16, name=f"wq{t}", tag="wq")
                   for t in range(DTn)]
            wks = [wkp.tile([P, D], bf16, name=f"wk{t}", tag="wk")
                   for t in range(DTn)]
            wvs = [wvp.tile([P, D], bf16, name=f"wv{t}", tag="wv")
                   for t in range(DTn)]
            wps = [wpp.tile([P, D], bf16, name=f"wp{t}", tag="wp")
                   for t in range(DTn)]
            for t in range(DTn):
                nc.sync.dma_start(xT[t][:], xT_d[t * P:(t + 1) * P, :])
            for tiles, src in ((wqs, wq_d), (wks, wk_d), (wvs, wv_d),
                               (wps, wp_d)):
                for t in range(DTn):
                    nc.sync.dma_start(tiles[t][:], src[t * P:(t + 1) * P, :])

            # ---- persistent result tiles ----
            qkT = [qkTp.tile([P, N], bf16, name=f"qkT{ft}", tag="qkT")
                   for ft in range(2 * NT)]
            vsb = [vp.tile([P, H * VW], bf16, name=f"v{jt}", tag="v")
                   for jt in range(NT)]
            attnT = [[attnp.tile([P, HC], bf16, name=f"attnT{c}_{q}",
                                 tag="attnT") for q in range(NT)]
                     for c in range(2)]

            # ---- emission task generators (PE filler work) ----
            def gen_qkT(ft):
                wt = wqs if ft < NT else wks
                col = (ft % NT) * P
                dst = qkT[ft]
                for ic in range(2):
                    ps = psEp.tile([P, HC], fp32, name="em_t", tag="em")
                    for dt in range(DTn):
                        nc.tensor.matmul(
                            ps[:], lhsT=wt[dt][:, col:col + P],
                            rhs=xT[dt][:, ic * HC:(ic + 1) * HC],
                            start=(dt == 0), stop=(dt == DTn - 1))
                        yield
                    nc.vector.tensor_copy(dst[:, ic * HC:(ic + 1) * HC],
                                          ps[:])
                    yield

            def gen_v(jt):
                v = vsb[jt]
                nc.gpsimd.memset(v[:], 1.0)
                vv = v[:].rearrange("p (h c) -> p h c", c=VW)
                for dc in range(2):
                    ps = psEp.tile([P, HC], fp32, name="em_t", tag="em")
                    for dt in range(DTn):
                        nc.tensor.matmul(
                            ps[:], lhsT=xT[dt][:, jt * P:(jt + 1) * P],
                            rhs=wvs[dt][:, dc * HC:(dc + 1) * HC],
                            start=(dt == 0), stop=(dt == DTn - 1))
                        yield
                    pv = ps[:].rearrange("p (h c) -> p h c", c=HD)
                    nc.vector.tensor_copy(vv[:, dc * 8:(dc + 1) * 8, 0:HD],
                                          pv)
                    yield

            def gen_proj(it):
                icb = it // 4
                ic2 = it % 4
                y = ysbp.tile([P, N], bf16, name="y_t", tag="y")
                for fc in range(2):
                    ps = psEp.tile([P, HC], fp32, name="em_t", tag="em")
                    for dt in range(DTn):
                        nc.tensor.matmul(
                            ps[:],
                            lhsT=attnT[icb][dt][:, ic2 * P:(ic2 + 1) * P],
                            rhs=wps[dt][:, fc * HC:(fc + 1) * HC],
                            start=(dt == 0), stop=(dt == DTn - 1))
                        yield
                    nc.vector.tensor_add(y[:, fc * HC:(fc + 1) * HC], ps[:],
                                         bias_bc[:, fc * HC:(fc + 1) * HC])
                    yield
                nc.sync.dma_start(out_d[it * P:(it + 1) * P, :], y[:])
                yield

            tasks = []

            def pump(n):
                while n > 0 and tasks:
                    try:
                        next(tasks[0])
                        n -= 1
                    except StopIteration:
                        tasks.pop(0)

            def drain():
                while tasks:
                    pump(1 << 30)

            # ---- attention stream ----
            e_of = {}
            av_of = {}

            def decode(g):
                icb, g2 = divmod(g, 64)
                hp, jt = divmod(g2, NT)
                return hp, icb, jt

            def SC(g):
                hp, icb, jt = decode(g)
                qa, ka = qkT[hp], qkT[NT + hp]
                i0 = icb * HC
                ps = psSp.tile([P, N], fp32, name="pss_t", tag="pss")
                nc.tensor.matmul(ps[:, 0:HC],
                                 lhsT=ka[0:HD, jt * P:(jt + 1) * P],
                                 rhs=qa[0:HD, i0:i0 + HC],
                                 start=True, stop=True)
                nc.tensor.matmul(ps[:, HC:N],
                                 lhsT=ka[HD:P, jt * P:(jt + 1) * P],
                                 rhs=qa[HD:P, i0:i0 + HC],
                                 start=True, stop=True)
                e = eTp.tile([P, N], bf16, name="e_t", tag="e")
                nc.scalar.activation(e[:], ps[:], EXP, scale=SCALE)
                e_of[g] = e

            def AV(g):
                hp, icb, jt = decode(g)
                ha, hb = 2 * hp, 2 * hp + 1
                if jt == 0:
                    av_of[hp] = {
                        h: psAp.tile([VW, HC], fp32, name=f"av{h}", tag="av")
                        for h in (ha, hb)
                    }
                e = e_of.pop(g)
                for h, ec in ((ha, 0), (hb, HC)):
                    nc.tensor.matmul(av_of[hp][h][:],
                                     lhsT=vsb[jt][:, h * VW:(h + 1) * VW],
                                     rhs=e[:, ec:ec + HC],
                                     start=(jt == 0), stop=(jt == NT - 1))

            def epilogue(hp, icb):
                ha, hb = 2 * hp, 2 * hp + 1
                for h, off in ((ha, 0), (hb, HD)):
                    avs = avsp.tile([VW, HC], bf16, name=f"avs{h}", tag="avs")
                    recf = recp.tile([1, HC], fp32, name=f"recf{h}",
                                     tag="recf")
                    nc.vector.reciprocal(recf[:], av_of[hp][h][HD:VW, :])
                    nc.vector.tensor_copy(avs[:], av_of[hp][h][:])
                    rb = rbp.tile([HD, HC], fp32, name=f"rb{h}", tag="rb")
                    nc.gpsimd.partition_broadcast(rb[:], recf[:])
                    nc.vector.tensor_mul(attnT[icb][hp][off:off + HD, :],
                                         avs[0:HD, :], rb[:])
                del av_of[hp]

            # pre-phase: q/k for pair 0, all of V (PE is DMA-gated here)
            tasks.append(gen_qkT(0))
            tasks.append(gen_qkT(NT))
            for jt in range(NT):
                tasks.append(gen_v(jt))
            drain()

            for p in range(1, NT):
                tasks.append(gen_qkT(p))
                tasks.append(gen_qkT(NT + p))

            SC(0)
            for g in range(128):
                hp, icb, jt = decode(g)
                if g + 1 < 128:
                    SC(g + 1)
                pump(6 if g < 64 else 3)
                AV(g)
                if jt == NT - 1:
                    epilogue(hp, icb)
                if g == 65:
                    for it in range(4):
                        tasks.append(gen_proj(it))
            for it in range(4, NT):
                tasks.append(gen_proj(it))
            drain()

    nc.compile()
    return nc


def get_nc():
    if "nc" not in _CACHE:
        _CACHE["nc"] = _build_nc()
    return _CACHE["nc"]


def make_in_maps(x, w_qkv, w_proj, b_proj):
    import ml_dtypes

    bf = ml_dtypes.bfloat16
    w = np.asarray(w_qkv, np.float32)
    wq = np.ascontiguousarray(w[:, 0:D]).astype(bf)
    wk = np.ascontiguousarray(w[:, D:2 * D]).astype(bf)
    wv = np.ascontiguousarray(w[:, 2 * D:3 * D]).astype(bf)
    wp = np.ascontiguousarray(np.asarray(w_proj, np.float32)).astype(bf)
    b2 = np.asarray(b_proj, np.float32).reshape(1, D).astype(bf)
    x = np.asarray(x, np.float32)
    maps = []
    for i in range(8):
        xT = np.ascontiguousarray(x[i].T).astype(bf)
        maps.append({"xT": xT, "wq": wq, "wk": wk, "wv": wv, "wp": wp,
                     "b_proj": b2})
    return maps


def kernel(x, w_qkv, w_proj, b_proj):
    from concourse import bass_utils

    nc = get_nc()
    in_maps = make_in_maps(x, w_qkv, w_proj, b_proj)
    res = bass_utils.run_bass_kernel_spmd(nc, in_maps, core_ids=list(range(8)))
    return np.stack(
        [np.asarray(res.results[i]["out"]).astype(np.float32)
         for i in range(8)], axis=0)
